# revision 41
# baseline (speedup 1.0000x reference)
"""NeRD pixel decoder (SIREN MLP over 5x5 local patches) on 8 trn2 cores.

Sharding: row-shard the pixel dim. Core c handles image b=c//4, rows
y0=(c%4)*32 .. y0+32 (4096 pixels). SIREN weights replicated.

Layer 0 (the 5x5 conv, 84% of FLOPs) runs in fp8-e4m3 DoubleRow matmuls at
0.5 cycles/row: per output row and 128-out-chan block, 25 taps are computed
as DR pairs (x_hi, x_lo) against stride-0-duplicated fp8 weights (x split
into hi + lo e4m3 parts on host, recovering ~11-bit input precision), plus 7
weight-residual correction DR pairs: 6 over vertically adjacent taps (their
windows don't overlap -- overlapping DR rhs windows crash the PE) and one
mixed pair whose halves are (w_lo of tap12) and the coords contribution
(gx/gy baked into a third slab plane at the same in-plane offset as tap12,
so the pair's two-dim stride is exactly 2*SLAB). The coords plane's idle
partitions 2-127 carry a (dy-2,dx-2)-shifted x_hi copy, so the same mixed
pair also corrects tap P2TAP there for free -- that plus the f32r head paid
for dropping the 7th pair (-1.7us of PE time at rel err 1.88e-2 vs the 2e-2
gate; see errsim.py for the host-side error simulator that validates such
trades without hardware). Layers 1/2 (tiles 0-6) run fp8-e4m3 DoubleRow:
the DVE splits each bf16 h tile into e4m3 hi/lo planes, and each 512-px
psum bank takes two 256-px chains of three DR matmuls. The head runs f32r
weights x f32r h2 (h2 stored f32r; 12-bit mantissa, 1 cycle/row at >=256
moving -- same speed as bf16, much tighter than bf16's 8 bits). NOTE:
matmul start=True zeroes the WHOLE psum bank, not just the written region,
so multi-region banks must start exactly once and stop exactly once.

Drain biases ride IN the psum: [1,128] bias-row x [1,npx] ones matmuls
preload b into each merged [128, 2*npx] bank (m0|m1 halves), letting ONE
activation (a 3D-AP write into h's two k-halves) cover both m-halves --
~190ns less ACT serialization per drain layer. The final 128-px chain
(l1 -> l2 -> head on px 3968-4096) is act-latency-bound (each PE->ACT->PE
roundtrip costs ~185+act+240ns), so its h2 lands in a compact bf16 tile
(hcb) and the last head runs bf16 at 1 cycle/row (f32r pays 4x under 256
moving); ready fillers (heads 3584/6*TP) sit at each PE-waits-on-ACT point.

Pipeline: ~13 input DMAs on the SP queue in first-use order; HWDGE and the
DMA engines are single global devices (transfers serialize; ~650ns
issue + 625 HWDGE + 650 DGE per DMA, +900ns sem-prop before consumers see
data), so w0h-m0 (the 1.1us transfer that gates the first L0 unit) goes
FIRST. Warmup matmuls cover the DMA lead-in and p-state ramp (full clock
needs ~3us of continuous execution). m=1 row units lag m=0 by two rows at
the start (their weight DMA lands later) and catch up via double-m1 steps
at i=16/i=20, so both halves finish at i=31 and the tail starts two
unit-times earlier. The t=0/t=1 L1-f8 blocks are delayed one odd-z so the
h0q split they consume is two units old (act->DVE-split latency is ~1.2us;
later blocks have l2+head work in front that covers it). Tile-7 work is
spread so i=31's ACT queue holds only the final chain's activations.

Everything is quantized host-side (e4m3 via ml_dtypes, f32r/bf16 rounding);
the device only multiplies exactly and accumulates in f32 PSUM. Weight
scale 2^12 and x scale 2^2 keep e4m3 operands in normal range; the
activation scale folds 2^-14 back out (sin(OMEGA*(z+b0)) via ACT bias).

Measured on the 8-core axon trn2 setup: TimelineSim 79747 ns (from the
81631 ns session-start baseline; sim matched HW within 2% on the 147030 ns
original), rel err 1.88e-2 vs the fp32 reference (gate 2e-2; deterministic
for the fixed seed-0 inputs). Remaining makespan structure: first L0 at
~4.6us (serialized w0h+xs transfers + 900ns sem), ~69us act-saturated
PE stream, then a ~6.2us structural tail (chain roundtrips + 625+650+900ns
DMA pipeline + ~700ns drains) -- all at their cost-model floors.
"""

import numpy as np
import ml_dtypes

FC = 128      # feature channels
P = 5         # patch
HID = 256
OUT = 3
OMEGA = 30.0
B, H, W = 2, 128, 128
NCORES = 8
ROWS = H // 4            # 32 image rows per core
NPIX = ROWS * W          # 4096 pixels per core
SLABR = ROWS + 4         # 36 slab rows (2 halo each side)
SLABW = W + 4            # 132 slab cols (2 pad each side)
SLAB = SLABR * SLABW     # 4752
TP = 512                 # pixels per L1/L2/head PSUM tile (= 4 image rows)
NT = NPIX // TP          # 8 tiles per core

E4 = ml_dtypes.float8_e4m3
SX = 4.0                 # x (slab/coords) pre-scale
SWT = 4096.0             # layer-0 weight pre-scale
SW12 = 64.0              # L1/L2 fp8 weight pre-scale
NWARM = 72               # warmup DR matmuls during DMA lead-in
# w_lo-corrected taps: 6 vertical pairs (12 taps) + tap12 in the mixed pair
# whose second half is the coords plane, which also carries a shifted x_hi
# copy on partitions 2-127 so the same pair corrects tap P2TAP there too.
WPAIRS = [(0, 0), (1, 0), (2, 0), (3, 0), (0, 1), (1, 1)]
P2TAP = 19               # tap (dy=3, dx=4) corrected via the coords plane
WLBLK = 7 * 256          # per-m w0l bytes: 7 DR pair blocks (6 pairs + mixed)
WTAIL = 11               # packed b1|b2|b3|w3(f32r, 6 cols) columns (f32)
WBC = 1546               # packed w1|w2|bias-rows|b3|w3 columns (bf16)

_BUILT = {}


def _build(structure="v8"):
    key = structure
    if key in _BUILT:
        return _BUILT[key]

    import concourse.tile as tile
    import concourse.mybir as mybir
    from concourse import bacc

    f32 = mybir.dt.float32
    f32r = mybir.dt.float32r
    fp8 = mybir.dt.float8e4
    Sin = mybir.ActivationFunctionType.Sin
    DR = mybir.MatmulPerfMode.DoubleRow

    nc = bacc.Bacc("TRN2", target_bir_lowering=False, debug=False)

    xs = nc.dram_tensor("xs", [128, 3 * SLAB], fp8, kind="ExternalInput").ap()
    w0h = nc.dram_tensor("w0h", [128, 6400], fp8,
                         kind="ExternalInput").ap()
    w0l = nc.dram_tensor("w0l", [128, 8 + 2 * WLBLK], fp8,
                         kind="ExternalInput").ap()
    wt = nc.dram_tensor("wt", [128, WTAIL], f32r,
                        kind="ExternalInput").ap()
    wb = nc.dram_tensor("wb", [128, WBC], mybir.dt.bfloat16,
                        kind="ExternalInput").ap()
    wq = nc.dram_tensor("wq", [128, 2048], fp8, kind="ExternalInput").ap()
    out = nc.dram_tensor("out", [OUT, NPIX], f32, kind="ExternalOutput").ap()

    with tile.TileContext(nc) as tc:
        with (
            tc.tile_pool(name="const", bufs=1) as cpool,
            tc.tile_pool(name="h", bufs=2) as hpool,
            tc.tile_pool(name="osb", bufs=1) as opool,
            tc.tile_pool(name="ps", bufs=8, space="PSUM") as pspool,
        ):
            # ---- SBUF tiles ----
            xs_t = cpool.tile([128, 3 * SLAB], fp8, tag="xs", name="xs_t")
            w0h_t = cpool.tile([128, 6400], fp8, tag="w0h", name="w0h_t")
            w0l_t = cpool.tile([128, 8 + 2 * WLBLK], fp8, tag="w0l",
                               name="w0l_t")
            wt_t = cpool.tile([128, WTAIL], f32r, tag="wt", name="wt_t")
            wb_t = cpool.tile([128, WBC], mybir.dt.bfloat16, tag="wb",
                              name="wb_t")
            wq_t = cpool.tile([128, 2048], fp8, tag="wq", name="wq_t")
            h0q = cpool.tile([128, 4 * NPIX], fp8, tag="h0q", name="h0q")
            h1q = cpool.tile([128, 4 * NPIX], fp8, tag="h1q", name="h1q")
            scr8 = cpool.tile([128, 256], fp8, tag="scr8", name="scr8")
            scro = cpool.tile([128, 128], f32, tag="scro", name="scro")
            ones_t = cpool.tile([1, 256], mybir.dt.bfloat16, tag="ones",
                                name="ones_t")
            hcb = cpool.tile([128, 256], mybir.dt.bfloat16, tag="hcb",
                             name="hcb")
            out_sb = opool.tile([OUT, NPIX], f32, tag="osb")

            # packed views
            b0_v = w0l_t[:, 0:8].bitcast(f32)          # [128, 2]
            w1_v = wb_t[:, 0:512]
            w2_v = wb_t[:, 512:1024]
            b1_v = wt_t[:, 0:2].bitcast(f32)
            b2_v = wt_t[:, 2:4].bitcast(f32)
            b3_v = wt_t[:][0:OUT, 4:5].bitcast(f32)    # [3, 1]
            w3_v = wt_t[:, 5:11]                       # [128, 6] f32r head
            # bias rows (partition 0) for merged-psum bias preload:
            # [b1m0|b1m1|b2m0|b2m1] at 1024+i*128; b3 row at 1536:1539
            brow = wb_t[:][0:1, 1024:1536]
            b3row = wb_t[:][0:1, 1536:1539]
            w3b_v = wb_t[:, 1540:1546]                 # bf16 w3, final chunk

            xs3 = xs.rearrange("p (s n) -> p s n", s=3)
            xst3 = xs_t[:].rearrange("p (s n) -> p s n", s=3)

            def slab_rows(ap3, r0, r1):
                return ap3[:, :, r0 * SLABW:r1 * SLABW]

            # ---- input DMAs: deadline order, two HWDGE queues ----
            nc.sync.dma_start(w0h_t[:, 0:3200], w0h[:, 0:3200])  # m0
            nc.sync.dma_start(xst3[:, 0:2, 0:6 * SLABW],
                              xs3[:, 0:2, 0:6 * SLABW])          # hi/lo r0-6
            nc.sync.dma_start(w0l_t[:, 0:8 + WLBLK],
                              w0l[:, 0:8 + WLBLK])               # b0 + m0
            nc.sync.dma_start(xst3[:, 2:3, 0:6 * SLABW],
                              xs3[:, 2:3, 0:6 * SLABW])          # coords r0-6
            nc.sync.dma_start(w0h_t[:, 3200:6400], w0h[:, 3200:6400])  # m1
            nc.sync.dma_start(w0l_t[:, 8 + WLBLK:], w0l[:, 8 + WLBLK:])
            nc.sync.dma_start(slab_rows(xst3, 6, 14), slab_rows(xs3, 6, 14))
            nc.sync.dma_start(slab_rows(xst3, 14, 22), slab_rows(xs3, 14, 22))
            nc.sync.dma_start(slab_rows(xst3, 22, 30), slab_rows(xs3, 22, 30))
            nc.sync.dma_start(slab_rows(xst3, 30, 36), slab_rows(xs3, 30, 36))
            nc.sync.dma_start(wq_t[:], wq[:])
            nc.sync.dma_start(wb_t[:], wb[:])
            nc.sync.dma_start(wt_t[:], wt[:])

            # ---- PE warmup on scratch data (p-state ramp during DMA) ----
            nc.vector.memset(scr8[:], 0.0)
            nc.vector.memset(ones_t[:], 1.0)
            scr3 = scr8[:].rearrange("p (s n) -> p s n", s=2)
            for i in range(NWARM):
                psw = pspool.tile([128, 128], f32, tag="ps", name=f"psw{i}")
                nc.tensor.matmul(psw[:], scr3, scr3, start=True, stop=True,
                                 perf_mode=DR)
                if i == NWARM - 1:
                    nc.scalar.activation(scro[:], psw[:], Sin,
                                         bias=b0_v[:, 0:1], scale=1.0)

            # ---- fused pipeline ----
            bf16 = mybir.dt.bfloat16
            h0 = hpool.tile([128, 2 * NPIX], bf16, tag="h", name="h0")
            h1 = hpool.tile([128, 2 * NPIX], bf16, tag="h", name="h1")
            h2 = opool.tile([128, 2 * NPIX], f32r, tag="h2", name="h2")
            act_scale = OMEGA / (SX * SWT)

            def w0h_blk(m, k):
                off = m * 3200 + k * 128
                a = w0h_t[:, off:off + 128].unsqueeze(1).copy()
                a.ap[1] = [0, 2]   # stride-0: same hi-weights for both halves
                return a

            def emit_l0_unit(m, y):
                ps = pspool.tile([128, 128], f32, tag="ps",
                                 name=f"ps_l0_{m}_{y}")
                for k in range(25):
                    dy, dx = divmod(k, 5)
                    off = (y + dy) * SLABW + dx
                    nc.tensor.matmul(ps[:], w0h_blk(m, k),
                                     xst3[:, 0:2, off:off + 128],
                                     start=(k == 0), stop=False, perf_mode=DR)
                for pi, (dx, q) in enumerate(WPAIRS):
                    blk = 8 + WLBLK * m + pi * 256
                    lhs = w0l_t[:, blk:blk + 256].rearrange(
                        "p (t c) -> p t c", t=2)
                    off = (y + 2 * q) * SLABW + dx
                    rhs = xst3[:, 0:1, off:off + 128].copy()
                    rhs.ap[1] = [SLABW, 2]       # taps (2q,dx), (2q+1,dx)
                    nc.tensor.matmul(ps[:], lhs, rhs, start=False,
                                     stop=False, perf_mode=DR)
                # mixed pair last: (w_lo of tap12) x window + wcp x coords
                # plane -- tap12's window offset equals the coords window's
                # in-plane offset, so the two-dim stride is exactly 2*SLAB
                blk = 8 + WLBLK * m + 6 * 256
                lhs = w0l_t[:, blk:blk + 256].rearrange(
                    "p (t c) -> p t c", t=2)
                off = (y + 2) * SLABW + 2
                rhs = xst3[:, 0:1, off:off + 128].copy()
                rhs.ap[1] = [2 * SLAB, 2]
                nc.tensor.matmul(ps[:], lhs, rhs, start=False, stop=True,
                                 perf_mode=DR)
                nc.scalar.activation(
                    h0[:, m * NPIX + y * 128:m * NPIX + (y + 1) * 128],
                    ps[:], Sin, bias=b0_v[:, m:m + 1], scale=act_scale)

            def emit_dense(lname, hin, hout, wl_v, bl_v, px0, npx):
                pss = [pspool.tile([128, npx], f32, tag="ps",
                                   name=f"ps_{lname}_{m}_{px0}")
                       for m in range(2)]
                for k in range(2):      # k-major: fresh k=1 read comes last
                    for m in range(2):
                        nc.tensor.matmul(
                            pss[m][:],
                            wl_v[:, (k * 2 + m) * 128:(k * 2 + m + 1) * 128],
                            hin[:, k * NPIX + px0:k * NPIX + px0 + npx],
                            start=(k == 0), stop=(k == 1))
                for m in range(2):
                    nc.scalar.activation(
                        hout[:, m * NPIX + px0:m * NPIX + px0 + npx],
                        pss[m][:], Sin, bias=bl_v[:, m:m + 1], scale=OMEGA)

            def emit_dense_merged(lname, hin, hout, wl_v, bi, px0, npx):
                # one [128, 2*npx] psum (m0|m1 halves), bias preloaded via
                # ones-matmuls so a SINGLE activation covers both m-halves
                ps = pspool.tile([128, 2 * npx], f32, tag="ps",
                                 name=f"ps_{lname}m_{px0}")
                for m in range(2):
                    # start=True zeroes the WHOLE bank: only the first mm
                    # starts, only the last stops
                    nc.tensor.matmul(
                        ps[:, m * npx:(m + 1) * npx],
                        brow[:, (bi * 2 + m) * 128:(bi * 2 + m + 1) * 128],
                        ones_t[:][0:1, 0:npx],
                        start=(m == 0), stop=False, skip_group_check=True)
                for k in range(2):      # k-major: fresh k=1 read comes last
                    for m in range(2):
                        nc.tensor.matmul(
                            ps[:, m * npx:(m + 1) * npx],
                            wl_v[:, (k * 2 + m) * 128:(k * 2 + m + 1) * 128],
                            hin[:, k * NPIX + px0:k * NPIX + px0 + npx],
                            start=False, stop=(k == 1 and m == 1),
                            skip_group_check=True)
                hv = hout[:].rearrange("p (k n) -> p k n", k=2)[
                    :, :, px0:px0 + npx]
                nc.scalar.activation(
                    hv, ps[:].rearrange("p (m n) -> p m n", m=2), Sin,
                    scale=OMEGA)

            def emit_split(hq_t, h_t, k, px0, npx):
                # hq layout: hi at k*4096+px, lo at 8192+k*4096+px
                hi = hq_t[:, k * NPIX + px0:k * NPIX + px0 + npx]
                lo = hq_t[:, 2 * NPIX + k * NPIX + px0:
                          2 * NPIX + k * NPIX + px0 + npx]
                hsl = h_t[:, k * NPIX + px0:k * NPIX + px0 + npx]
                nc.vector.tensor_copy(hi, hsl)
                nc.vector.tensor_sub(lo, hsl, hi)

            def emit_dense_f8(lname, loff, hq_t, hout, bl_v, t):
                # one [128,512] psum bank per m; two 256-px DR chains each
                pss = [pspool.tile([128, TP], f32, tag="ps",
                                   name=f"ps_{lname}f8_{m}_{t}")
                       for m in range(2)]

                def hi_rhs(k, off):
                    a = hq_t[:, k * NPIX + off:k * NPIX + off + 256]
                    a = a.unsqueeze(1).copy()
                    a.ap[1] = [2 * NPIX, 2]        # (hi_k, lo_k)
                    return a

                def wc_rhs(off):
                    a = hq_t[:, off:off + 256].unsqueeze(1).copy()
                    a.ap[1] = [NPIX, 2]            # (hi_k0, hi_k1)
                    return a

                def lhs_main(k, m):
                    a = wq_t[:, loff + (k * 2 + m) * 128:
                             loff + (k * 2 + m) * 128 + 128]
                    a = a.unsqueeze(1).copy()
                    a.ap[1] = [0, 2]
                    return a

                for sub in (0, 256):
                    off = t * TP + sub
                    for m in range(2):
                        o = pss[m][:, sub:sub + 256]
                        nc.tensor.matmul(o, lhs_main(0, m), hi_rhs(0, off),
                                         start=(sub == 0), stop=False,
                                         perf_mode=DR, skip_group_check=True)
                        nc.tensor.matmul(o, lhs_main(1, m), hi_rhs(1, off),
                                         start=False, stop=False,
                                         perf_mode=DR, skip_group_check=True)
                        lw = wq_t[:, loff + 512 + m * 256:
                                  loff + 512 + (m + 1) * 256].rearrange(
                            "p (t c) -> p t c", t=2)
                        nc.tensor.matmul(o, lw, wc_rhs(off), start=False,
                                         stop=(sub == 256), perf_mode=DR,
                                         skip_group_check=True)
                for m in range(2):
                    nc.scalar.activation(
                        hout[:, m * NPIX + t * TP:m * NPIX + (t + 1) * TP],
                        pss[m][:], Sin, bias=bl_v[:, m:m + 1],
                        scale=OMEGA / SW12)

            def emit_head(px0, npx):
                # f32r weights x f32r h2: 1 cycle/row at npx >= 256
                ps = pspool.tile([OUT, npx], f32, tag="ps",
                                 name=f"ps_hd_{px0}")
                for k in range(2):
                    nc.tensor.matmul(
                        ps[:], w3_v[:, k * OUT:(k + 1) * OUT],
                        h2[:, k * NPIX + px0:k * NPIX + px0
                           + npx],
                        start=(k == 0), stop=(k == 1))
                nc.vector.tensor_scalar_add(
                    out_sb[:, px0:px0 + npx], ps[:], b3_v)

            def out_dma(px0, px1, last=False):
                nc.sync.dma_start(out[:, px0:px1], out_sb[:, px0:px1])

            def l1(px0, npx):
                emit_dense("l1", h0, h1, w1_v, b1_v, px0, npx)

            def l2(px0, npx):
                emit_dense("l2", h1, h2, w2_v, b2_v, px0, npx)

            # m1 lags m0 by 2 rows at the start (its weight DMA lands later)
            # and catches up via double-m1 steps at i=16/i=20, so both
            # m-halves finish at i=31 and the drain tail starts ~2 unit-times
            # earlier.
            def m1_z_for_step(i):
                if i < 2:
                    return []
                if i < 16:
                    return [i - 2]
                if i == 16:
                    return [14, 15]
                if i < 20:
                    return [i - 1]
                if i == 20:
                    return [19, 20]
                return [i]

            for i in range(ROWS):
                emit_l0_unit(0, i)
                if i % 2 == 1 and i < 28:
                    emit_split(h0q, h0, 0, (i - 1) * 128, 256)
                for z in m1_z_for_step(i):
                    emit_l0_unit(1, z)
                    if z % 2 == 1 and z < 28:
                        emit_split(h0q, h0, 1, (z - 1) * 128, 256)
                    # t=0/t=1 blocks are delayed one odd-z so the h0q split
                    # they consume is two units old (no PE-waits-DVE stall);
                    # t>=2 blocks have l2+head work in front, which covers it
                    t = {5: 0, 9: 1, 11: 2, 15: 3, 19: 4, 23: 5, 27: 6}.get(z)
                    if t is not None:
                        if t >= 1:
                            emit_dense_f8("l2", 1024, h1q, h2, b2_v, t - 1)
                        if t >= 2:
                            emit_head((t - 2) * TP, TP)
                            if t in (3, 5, 7):
                                out_dma((t - 3) * TP, (t - 1) * TP)
                        emit_dense_f8("l1", 0, h0q, h1, b1_v, t)
                        for m in range(2):
                            emit_split(h1q, h1, m, t * TP, TP)
                    elif z == 29:
                        # tile-7 front: everything that only needs rows <= 29
                        emit_head(5 * TP, TP)
                        emit_dense_merged("l1", h0, h1, w1_v, 0, 3584, 256)
                        out_dma(4 * TP, 6 * TP)
                    elif z == 30:
                        # tile-7 mid: keep all non-chain activations in this
                        # step so i=31's ACT queue holds only the final chain;
                        # l2_f8(6) first: it has no fresh deps, so it fills
                        # the PE window while y30/z30 acts run
                        emit_dense_f8("l2", 1024, h1q, h2, b2_v, 6)
                        emit_dense_merged("l1", h0, h1, w1_v, 0, 3840, 128)
                        emit_dense_merged("l2", h1, h2, w2_v, 1, 3584, 256)
                        emit_dense_merged("l2", h1, h2, w2_v, 1, 3840, 128)
                    elif z == 31:
                        # final 128-px chain l1 -> l2 -> head on px 3968-4096,
                        # hand-rolled k-major with ready fillers at each
                        # PE-waits-on-ACT point
                        px0 = 3968
                        ps1 = pspool.tile([128, 256], f32, tag="ps",
                                          name="ps_l1c")
                        ps2 = pspool.tile([128, 256], f32, tag="ps",
                                          name="ps_l2c")

                        def chain_k(ps, wl_v, hin, k, bias=False):
                            for m in range(2):
                                if bias and k == 0:
                                    nc.tensor.matmul(
                                        ps[:, m * 128:(m + 1) * 128],
                                        brow[:, (bias * 2 + m - 2) * 128:
                                             (bias * 2 + m - 1) * 128],
                                        ones_t[:][0:1, 0:128],
                                        start=(m == 0), stop=False,
                                        skip_group_check=True)
                                nc.tensor.matmul(
                                    ps[:, m * 128:(m + 1) * 128],
                                    wl_v[:, (k * 2 + m) * 128:
                                         (k * 2 + m + 1) * 128],
                                    hin[:, k * NPIX + px0:k * NPIX + px0
                                        + 128],
                                    start=False, stop=(k == 1 and m == 1),
                                    skip_group_check=True)

                        def chain_act(ps, hout):
                            if hout is hcb:
                                nc.scalar.activation(hcb[:], ps[:], Sin,
                                                     scale=OMEGA)
                                return
                            hv = hout[:].rearrange("p (k n) -> p k n", k=2)[
                                :, :, px0:px0 + 128]
                            nc.scalar.activation(
                                hv, ps[:].rearrange("p (m n) -> p m n", m=2),
                                Sin, scale=OMEGA)

                        def head_k(ps, k, px0h, npx):
                            nc.tensor.matmul(
                                ps[:], w3_v[:, k * OUT:(k + 1) * OUT],
                                h2[:, k * NPIX + px0h:k * NPIX + px0h
                                   + npx],
                                start=(k == 0), stop=(k == 1))

                        def head_k3(ps, k, px0h, npx):
                            # accumulates onto the b3 preload
                            nc.tensor.matmul(
                                ps[:], w3_v[:, k * OUT:(k + 1) * OUT],
                                h2[:, k * NPIX + px0h:k * NPIX + px0h
                                   + npx],
                                start=False, stop=(k == 1),
                                skip_group_check=True)

                        ps6 = pspool.tile([OUT, TP], f32, tag="ps",
                                          name="ps_hd6")
                        psa = pspool.tile([OUT, 128], f32, tag="ps",
                                          name="ps_hda")
                        psb = pspool.tile([OUT, 128], f32, tag="ps",
                                          name="ps_hdb")

                        def bias3(ps, npx):
                            nc.tensor.matmul(ps[:], b3row,
                                             ones_t[:][0:1, 0:npx],
                                             start=True, stop=False,
                                             skip_group_check=True)

                        # head-a (3840-3968) only needs i=30 data: pure filler
                        bias3(psa, 128)
                        chain_k(ps1, w1_v, h0, 0, bias=1)  # needs act(y31 m0)
                        head_k3(psa, 0, 3840, 128)
                        chain_k(ps1, w1_v, h0, 1)
                        head_k3(psa, 1, 3840, 128)
                        nc.vector.tensor_copy(out_sb[:, 3840:3968], psa[:])
                        chain_act(ps1, h1)
                        emit_head(3584, 256)          # filler over act(h1c)
                        chain_k(ps2, w2_v, h1, 0, bias=2)
                        head_k(ps6, 0, 6 * TP, TP)
                        chain_k(ps2, w2_v, h1, 1)
                        head_k(ps6, 1, 6 * TP, TP)
                        nc.vector.tensor_scalar_add(
                            out_sb[:, 6 * TP:7 * TP], ps6[:], b3_v)
                        chain_act(ps2, hcb)
                        out_dma(6 * TP, 3968)
                        # final 128-px head reads the bf16 chain tile (1
                        # cycle/row; f32r would pay 4x under 256 moving)
                        bias3(psb, 128)
                        for k in range(2):
                            nc.tensor.matmul(
                                psb[:], w3b_v[:, k * OUT:(k + 1) * OUT],
                                hcb[:, k * 128:(k + 1) * 128],
                                start=False, stop=(k == 1),
                                skip_group_check=True)
                        nc.vector.tensor_copy(out_sb[:, 3968:NPIX], psb[:])
                        out_dma(3968, NPIX, last=True)

    nc.finalize()
    _BUILT[key] = nc
    return nc


def _to_f32r(a):
    """Round fp32 to the fp32r format the PE expects (low 12 mantissa bits 0)."""
    b = np.ascontiguousarray(a, np.float32).view(np.uint32).astype(np.uint64)
    r = ((b + 0x800) & 0xFFFFF000).astype(np.uint32)
    return r.view(np.float32).reshape(np.asarray(a).shape)


def _e4(a):
    return np.ascontiguousarray(a, np.float32).astype(E4)


def _prep_core_inputs(c, xi, gx, gy):
    b = c // 4
    y0 = (c % 4) * ROWS
    slab = np.zeros((128, SLABR, SLABW), np.float32)
    ylo, yhi = y0 - 2, y0 + ROWS + 2
    slo, shi = max(ylo, 0), min(yhi, H)
    slab[:, slo - ylo: shi - ylo, 2:2 + W] = xi[b, :, slo:shi, :]
    slab *= SX
    xh = _e4(slab)
    xl = _e4(slab - xh.astype(np.float32))

    csl = np.zeros((128, SLABR, SLABW), np.float32)
    # partitions 2-127: x_hi shifted by tap P2TAP's (dy-2, dx-2)=(1, 2) so
    # the mixed pair's coords-plane half also corrects that tap there
    csl[2:, 0:SLABR - 1, 0:SLABW - 2] = xh[2:, 1:, 2:].astype(np.float32)
    csl[0, 2:35, 2:130] = SX * gx[None, :]
    # gy per slab row r (used at window row y'+2 -> image row y0+y'):
    for r in range(2, 35):
        csl[1, r, 2:130] = SX * gy[min(max(y0 + r - 2, 0), H - 1)]

    return {
        "xs": np.concatenate(
            [xh.reshape(128, SLAB), xl.reshape(128, SLAB),
             _e4(csl.reshape(128, SLAB))], axis=1),
    }


def kernel(**inputs):
    from concourse.bass_utils import run_bass_kernel_spmd

    xi = np.asarray(inputs["xi"], np.float32)
    W0 = np.asarray(inputs["W0"], np.float32)
    b0 = np.asarray(inputs["b0"], np.float32)
    W1 = np.asarray(inputs["W1"], np.float32)
    b1 = np.asarray(inputs["b1"], np.float32)
    W2 = np.asarray(inputs["W2"], np.float32)
    b2 = np.asarray(inputs["b2"], np.float32)
    W3 = np.asarray(inputs["W3"], np.float32)
    b3 = np.asarray(inputs["b3"], np.float32)

    # ---- weight prep (replicated) ----
    # patch rows of W0 are (c, dy, dx)-ordered; ktile k=(dy*5+dx) gathers
    # rows c*25+k. Scale 2^12, split hi/lo in e4m3.
    Wp = (SWT * W0[:FC * P * P]).reshape(128, 25, HID)   # [c, k, out]
    wh_f = _e4(Wp).astype(np.float32)
    wl_f = Wp - wh_f
    # coords weight pad: [m][2 halves][128]; half0 rows 0,1 = SWT*Wc
    Wc = SWT * W0[FC * P * P:]                            # [2, 256]
    wcp = np.zeros((128, 2, 2, 128), np.float32)
    for m in range(2):
        wcp[0:2, m, 0, :] = Wc[:, m * 128:(m + 1) * 128]
    # w0h: [m=0 taps][m=1 taps]
    w0h_pk = np.empty((128, 6400), np.float32)
    for m in range(2):
        for k in range(25):
            off = m * 3200 + k * 128
            w0h_pk[:, off:off + 128] = wh_f[:, k, m * 128:(m + 1) * 128]
    # w0l: [b0(8 bytes)][m=0 pair blocks][m=1 pair blocks]
    b0_h = np.ascontiguousarray((OMEGA * b0).reshape(2, 128).T,
                                np.float32)               # [128, 2]
    w0l_pk = np.zeros((128, 8 + 2 * WLBLK), E4)
    w0l_pk[:, 0:8] = b0_h.view(np.uint8).reshape(128, 8).view(E4)
    for m in range(2):
        for pi, (dx, q) in enumerate(WPAIRS):
            for j in range(2):
                k = (2 * q + j) * 5 + dx
                off = 8 + WLBLK * m + pi * 256 + j * 128
                w0l_pk[:, off:off + 128] = _e4(
                    wl_f[:, k, m * 128:(m + 1) * 128])
        off = 8 + WLBLK * m + 6 * 256
        w0l_pk[:, off:off + 128] = _e4(wl_f[:, 12, m * 128:(m + 1) * 128])
        # mixed-pair second half: coords weights on rows 0-1, w_lo of tap
        # P2TAP on rows 2-127 (paired with the shifted x_hi in the coords
        # plane)
        w0l_pk[:, off + 128:off + 256] = _e4(wcp[:, m, 0, :])
        w0l_pk[2:, off + 128:off + 256] = _e4(
            wl_f[2:, P2TAP, m * 128:(m + 1) * 128])

    # wb: [w1|w2|bias rows] bf16; wt: [b1|b2|b3|w3(f32r)] f32
    wb_pk = np.zeros((128, WBC), ml_dtypes.bfloat16)
    wb_pk[:, 0:512] = W1.reshape(2, 128, 2, 128).transpose(
        1, 0, 2, 3).reshape(128, 512).astype(ml_dtypes.bfloat16)
    wb_pk[:, 512:1024] = W2.reshape(2, 128, 2, 128).transpose(
        1, 0, 2, 3).reshape(128, 512).astype(ml_dtypes.bfloat16)
    # raw-b rows on partition 0 for merged-psum bias preload (act scale
    # multiplies by OMEGA afterwards)
    wb_pk[0, 1024:1280] = b1.astype(ml_dtypes.bfloat16)
    wb_pk[0, 1280:1536] = b2.astype(ml_dtypes.bfloat16)
    wb_pk[0, 1536:1539] = b3.astype(ml_dtypes.bfloat16)
    wb_pk[:, 1540:1546] = W3.reshape(2, 128, OUT).transpose(
        1, 0, 2).reshape(128, 2 * OUT).astype(ml_dtypes.bfloat16)
    wt_pk = np.zeros((128, WTAIL), np.float32)
    wt_pk[:, 0:2] = np.ascontiguousarray((OMEGA * b1).reshape(2, 128).T)
    wt_pk[:, 2:4] = np.ascontiguousarray((OMEGA * b2).reshape(2, 128).T)
    wt_pk[0:OUT, 4] = b3
    wt_pk[:, 5:11] = _to_f32r(W3.reshape(2, 128, OUT).transpose(
        1, 0, 2).reshape(128, 2 * OUT))

    ys = np.linspace(-1.0, 1.0, H, dtype=np.float32)
    xcs = np.linspace(-1.0, 1.0, W, dtype=np.float32)

    # wq: per layer [hi blocks (k,m)][lo pairs (m)] in e4m3, scale 2^6
    wq_pk = np.zeros((128, 2048), E4)
    for li, Wl in ((0, W1), (1, W2)):
        whf = _e4(SW12 * Wl).astype(np.float32)
        wlf = SW12 * Wl - whf
        base = li * 1024
        for k in range(2):
            for m in range(2):
                off = base + (k * 2 + m) * 128
                wq_pk[:, off:off + 128] = _e4(
                    whf[k * 128:(k + 1) * 128, m * 128:(m + 1) * 128])
        for m in range(2):
            for k in range(2):
                off = base + 512 + m * 256 + k * 128
                wq_pk[:, off:off + 128] = _e4(
                    wlf[k * 128:(k + 1) * 128, m * 128:(m + 1) * 128])

    shared = {"w0h": _e4(w0h_pk), "w0l": w0l_pk, "wt": wt_pk,
              "wb": wb_pk, "wq": wq_pk}
    in_maps = []
    for c in range(NCORES):
        m = _prep_core_inputs(c, xi, xcs, ys)
        m.update(shared)
        in_maps.append(m)

    nc = _build()
    res = run_bass_kernel_spmd(nc, in_maps, core_ids=list(range(NCORES)))
    global LAST_RES
    LAST_RES = res

    full = np.empty((B, OUT, H, W), np.float32)
    for c in range(NCORES):
        b = c // 4
        y0 = (c % 4) * ROWS
        full[b, :, y0:y0 + ROWS, :] = res.results[c]["out"].reshape(
            OUT, ROWS, W)
    return full

